# revision 2
# baseline (speedup 1.0000x reference)
import numpy as np

import concourse.bacc as bacc
import concourse.mybir as mybir
import concourse.tile as tile
from concourse.bass_utils import run_bass_kernel_spmd

F32 = mybir.dt.float32
F32R = mybir.dt.float32r
AF = mybir.ActivationFunctionType

B, S, D, H, DK = 4, 1024, 1024, 16, 64
NHEAD = 8          # heads per core
GJ = 512           # head-dim columns per core's head group
NEG = -8.0e4       # pre-scale mask addend; *0.125 -> -1e4 -> exp underflows to 0.0


def _build(mode: str, repeat: int = 1):
    nc = bacc.Bacc("TRN2", target_bir_lowering=False)

    XQ = nc.dram_tensor("XQ", [D, S], F32, kind="ExternalInput")
    XK = nc.dram_tensor("XK", [D, S], F32, kind="ExternalInput")
    XV = nc.dram_tensor("XV", [D, S], F32, kind="ExternalInput")
    WQ = nc.dram_tensor("WQ", [D, GJ], F32, kind="ExternalInput")
    WK = nc.dram_tensor("WK", [D, GJ], F32, kind="ExternalInput")
    WV = nc.dram_tensor("WV", [D, GJ], F32, kind="ExternalInput")
    WOT = nc.dram_tensor("WOT", [GJ, D], F32, kind="ExternalInput")
    BQ = nc.dram_tensor("BQ", [1, GJ], F32, kind="ExternalInput")
    BK = nc.dram_tensor("BK", [1, GJ], F32, kind="ExternalInput")
    ONESR = nc.dram_tensor("ONESR", [1, GJ], F32, kind="ExternalInput")
    IDENT = nc.dram_tensor("IDENT", [128, 128], F32, kind="ExternalInput")
    ZEROS = nc.dram_tensor("ZEROS", [128, 128], F32, kind="ExternalInput")
    MASKP = MASKF = None
    if mode == "causal":
        MASKP = nc.dram_tensor("MASKP", [128, 4 * 512], F32, kind="ExternalInput")
    elif mode == "generic":
        MASKF = nc.dram_tensor("MASKF", [S, S], F32, kind="ExternalInput")

    ATT = nc.dram_tensor("ATT", [NHEAD, S, S], F32, kind="ExternalOutput")
    OUTP = nc.dram_tensor("OUTP", [S, D], F32, kind="ExternalOutput")

    causal = mode == "causal"

    with tile.TileContext(nc) as tc:
        with (
            tc.tile_pool(name="pers", bufs=1) as pp,
            tc.tile_pool(name="psum", bufs=1, space="PSUM") as ps,
        ):
            qt = [pp.tile([128, S], F32R, name=f"qt{t}") for t in range(4)]
            kt = [pp.tile([128, S], F32R, name=f"kt{t}") for t in range(4)]
            v1 = [pp.tile([128, GJ], F32R, name=f"v1{j}") for j in range(8)]
            ct = [pp.tile([128, S], F32R, name=f"ct{t}") for t in range(4)]
            wot = [pp.tile([128, S], F32R, name=f"wot{e}") for e in range(4)]
            idr = pp.tile([128, 128], F32R, name="idr")
            idf = pp.tile([128, 128], F32, name="idf")
            zrf = pp.tile([128, 128], F32, name="zrf")
            onesr = pp.tile([1, GJ], F32R, name="onesr")
            bq_t = pp.tile([1, GJ], F32R, name="bq_t")
            bk_t = pp.tile([1, GJ], F32R, name="bk_t")
            mkp = None
            mkf = None
            if causal:
                mkp = pp.tile([128, 4 * 512], F32R, name="mkp")
            elif mode == "generic":
                mkf = [pp.tile([128, S], F32R, name=f"mkf{it}") for it in range(8)]

            for rep in range(repeat):
                R = f"r{rep}_"
                for e in range(4):
                    nc.sync.dma_start(wot[e], WOT[e * 128:(e + 1) * 128, :].bitcast(F32R))
                nc.sync.dma_start(idr, IDENT[:, :].bitcast(F32R))
                nc.sync.dma_start(idf, IDENT[:, :])
                nc.sync.dma_start(zrf, ZEROS[:, :])
                nc.sync.dma_start(onesr, ONESR[:, :].bitcast(F32R))
                nc.sync.dma_start(bq_t, BQ[:, :].bitcast(F32R))
                nc.sync.dma_start(bk_t, BK[:, :].bitcast(F32R))
                if causal:
                    nc.sync.dma_start(mkp, MASKP[:, :].bitcast(F32R))
                elif mode == "generic":
                    for it in range(8):
                        nc.sync.dma_start(mkf[it], MASKF[it * 128:(it + 1) * 128, :].bitcast(F32R))

                # ---------------- projections ----------------
                with tc.tile_pool(name=f"{R}proj", bufs=1) as sx:
                    for (xd, wd, btile, kind) in (
                        (XQ, WQ, bq_t, "q"), (XK, WK, bk_t, "k"), (XV, WV, None, "v")
                    ):
                        xc, wc = [], []
                        for kc in range(8):
                            xt = sx.tile([128, S], F32R, tag=f"x{kc}", name=f"{R}x_{kind}{kc}")
                            nc.sync.dma_start(xt, xd[kc * 128:(kc + 1) * 128, :].bitcast(F32R))
                            xc.append(xt)
                            wt = sx.tile([128, GJ], F32R, tag=f"w{kc}", name=f"{R}w_{kind}{kc}")
                            nc.sync.dma_start(wt, wd[kc * 128:(kc + 1) * 128, :].bitcast(F32R))
                            wc.append(wt)
                        if kind in ("q", "k"):
                            dst = qt if kind == "q" else kt
                            for jt in range(4):
                                for ih in range(2):
                                    p = ps.tile([128, 512], F32, tag="mm", bufs=3,
                                                name=f"{R}p{kind}{jt}{ih}")
                                    for kc in range(8):
                                        nc.tensor.matmul(
                                            p, wc[kc][:, jt * 128:(jt + 1) * 128],
                                            xc[kc][:, ih * 512:(ih + 1) * 512],
                                            start=(kc == 0), stop=False)
                                    nc.tensor.matmul(
                                        p, btile[0:1, jt * 128:(jt + 1) * 128], onesr,
                                        start=False, stop=True)
                                    nc.scalar.copy(dst[jt][:, ih * 512:(ih + 1) * 512], p)
                        else:
                            for st in range(8):
                                p = ps.tile([128, 512], F32, tag="mm", bufs=3, name=f"{R}pv{st}")
                                for kc in range(8):
                                    nc.tensor.matmul(
                                        p, xc[kc][:, st * 128:(st + 1) * 128], wc[kc],
                                        start=(kc == 0), stop=(kc == 7))
                                nc.scalar.copy(v1[st], p)

                # ---------------- attention ----------------
                with tc.tile_pool(name=f"{R}work", bufs=1) as sw:
                    for h in range(NHEAD):
                        th, off = h // 2, 64 * (h % 2)
                        attb = {}
                        for ih in range(2):
                            for it in range(4 * ih, 4 * ih + 4):
                                jhs = [0, 1] if (it >= 4 or not causal) else [0]
                                accs, exps = [], {}
                                for jh in jhs:
                                    sp = ps.tile([128, 512], F32, tag="mm", bufs=3,
                                                 name=f"{R}s{h}_{it}_{jh}")
                                    qv = it - 4 * jh
                                    mask_add = (mode == "generic") or (causal and 0 <= qv <= 3)
                                    nc.tensor.matmul(
                                        sp,
                                        qt[th][off:off + 64, it * 128:(it + 1) * 128],
                                        kt[th][off:off + 64, jh * 512:(jh + 1) * 512],
                                        start=True, stop=not mask_add)
                                    if mask_add:
                                        rhs = (mkp[:, qv * 512:(qv + 1) * 512] if causal
                                               else mkf[it][:, jh * 512:(jh + 1) * 512])
                                        nc.tensor.matmul(sp, idr, rhs, start=False, stop=True)
                                    ex = sw.tile([128, 512], F32R, tag=f"ex{jh}", bufs=2,
                                                 name=f"{R}ex{h}_{it}_{jh}")
                                    ac = sw.tile([128, 1], F32, tag=f"ac{jh}", bufs=2,
                                                 name=f"{R}ac{h}_{it}_{jh}")
                                    nc.scalar.activation(ex, sp, AF.Exp, bias=0.0, scale=0.125,
                                                         accum_out=ac[:, 0:1])
                                    exps[jh] = ex
                                    accs.append(ac)
                                rec = sw.tile([128, 1], F32, tag="rec", bufs=4, name=f"{R}rc{h}_{it}")
                                if len(accs) == 2:
                                    sm = sw.tile([128, 1], F32, tag="sm", bufs=2, name=f"{R}sm{h}_{it}")
                                    nc.vector.tensor_add(sm, accs[0], accs[1])
                                    nc.vector.reciprocal(rec, sm)
                                else:
                                    nc.vector.reciprocal(rec, accs[0])
                                for jh in jhs:
                                    at = sw.tile([128, 512], F32, tag=f"at{it}_{jh}",
                                                 name=f"{R}at{h}_{it}_{jh}")
                                    nc.scalar.activation(at, exps[jh], AF.Identity, bias=0.0,
                                                         scale=rec[:, 0:1])
                                    attb[(it, jh)] = at
                                    qv = it - 4 * jh
                                    W = (qv + 1) * 128 if (causal and 0 <= qv <= 3) else 512
                                    nc.sync.dma_start(
                                        ATT[h:h + 1, it * 128:(it + 1) * 128, jh * 512:jh * 512 + W],
                                        at[:, 0:W])
                            # ---- AV for this query half ----
                            jts = list(range(4 + 4 * ih)) if causal else list(range(8))
                            up = ps.tile([64, 512], F32, tag="u", bufs=2, name=f"{R}u{h}_{ih}")
                            for jn, jt in enumerate(jts):
                                jh, js = jt // 4, jt % 4
                                tp = ps.tile([128, 512], F32, tag="tr", bufs=2,
                                             name=f"{R}tp{h}_{ih}_{jt}")
                                for idx, it in enumerate(range(4 * ih, 4 * ih + 4)):
                                    if (not causal) or it >= jt:
                                        src = attb[(it, jh)][:, js * 128:(js + 1) * 128]
                                    else:
                                        src = zrf[:, :]
                                    nc.tensor.transpose(tp[:, idx * 128:(idx + 1) * 128], src, idf)
                                aT = sw.tile([128, 512], F32R, tag="aT", bufs=2,
                                             name=f"{R}aT{h}_{ih}_{jt}")
                                nc.vector.tensor_copy(aT, tp)
                                nc.tensor.matmul(up, v1[jt][:, h * 64:(h + 1) * 64], aT,
                                                 start=(jn == 0), stop=(jn == len(jts) - 1))
                            nc.scalar.copy(ct[th][off:off + 64, ih * 512:(ih + 1) * 512], up)

                    # ---------------- output projection ----------------
                    for ic in range(8):
                        for dh in range(2):
                            op = ps.tile([128, 512], F32, tag="mm", bufs=3, name=f"{R}o{ic}_{dh}")
                            for ec in range(4):
                                nc.tensor.matmul(
                                    op, ct[ec][:, ic * 128:(ic + 1) * 128],
                                    wot[ec][:, dh * 512:(dh + 1) * 512],
                                    start=(ec == 0), stop=(ec == 3))
                            ob = sw.tile([128, 512], F32, tag="ob", bufs=2, name=f"{R}ob{ic}_{dh}")
                            nc.scalar.copy(ob, op)
                            nc.sync.dma_start(
                                OUTP[ic * 128:(ic + 1) * 128, dh * 512:(dh + 1) * 512], ob)
    nc.finalize()
    return nc


def _causal_mask_patterns() -> np.ndarray:
    ir = np.arange(128)[:, None]
    jr = np.arange(512)[None, :]
    pats = [np.where(jr > 128 * q + ir, NEG, 0.0).astype(np.float32) for q in range(4)]
    return np.ascontiguousarray(np.concatenate(pats, axis=1))


def _detect_mode(m2: np.ndarray) -> str:
    if not m2.any():
        return "dense"
    if np.array_equal(m2, np.triu(np.ones((S, S), dtype=m2.dtype), 1)):
        return "causal"
    return "generic"


_NC_CACHE: dict = {}
_LAST = {}


def _make_in_maps(mode, query, key, value, m2, Wq, bq, Wk, bk, Wv, Wo):
    ones = np.ones((1, GJ), dtype=np.float32)
    eye = np.eye(128, dtype=np.float32)
    zeros = np.zeros((128, 128), dtype=np.float32)
    maskp = _causal_mask_patterns() if mode == "causal" else None
    maskf = np.ascontiguousarray((m2 * np.float32(NEG)).astype(np.float32)) if mode == "generic" else None
    in_maps = []
    for c in range(8):
        b, g = divmod(c, 2)
        gs = slice(GJ * g, GJ * (g + 1))
        im = {
            "XQ": np.ascontiguousarray(query[b].T.astype(np.float32, copy=False)),
            "XK": np.ascontiguousarray(key[b].T.astype(np.float32, copy=False)),
            "XV": np.ascontiguousarray(value[b].T.astype(np.float32, copy=False)),
            "WQ": np.ascontiguousarray(Wq[gs, :].T.astype(np.float32, copy=False)),
            "WK": np.ascontiguousarray(Wk[gs, :].T.astype(np.float32, copy=False)),
            "WV": np.ascontiguousarray(Wv[gs, :].T.astype(np.float32, copy=False)),
            "WOT": np.ascontiguousarray(Wo[:, gs].T.astype(np.float32, copy=False)),
            "BQ": np.ascontiguousarray(bq[gs].reshape(1, GJ).astype(np.float32, copy=False)),
            "BK": np.ascontiguousarray(bk[gs].reshape(1, GJ).astype(np.float32, copy=False)),
            "ONESR": ones,
            "IDENT": eye,
            "ZEROS": zeros,
        }
        if mode == "causal":
            im["MASKP"] = maskp
        elif mode == "generic":
            im["MASKF"] = maskf
        in_maps.append(im)
    return in_maps


def kernel(query, key, value, mask, Wq, bq, Wk, bk, Wv, bv, Wo, bo):
    query = np.asarray(query)
    key = np.asarray(key)
    value = np.asarray(value)
    mask = np.asarray(mask)
    Wq, bq = np.asarray(Wq), np.asarray(bq)
    Wk, bk = np.asarray(Wk), np.asarray(bk)
    Wv, bv = np.asarray(Wv), np.asarray(bv)
    Wo, bo = np.asarray(Wo), np.asarray(bo)

    m2 = mask.reshape(S, S).astype(np.float32, copy=False)
    mode = _detect_mode(m2)
    if mode not in _NC_CACHE:
        _NC_CACHE[mode] = _build(mode)
    nc = _NC_CACHE[mode]

    in_maps = _make_in_maps(mode, query, key, value, m2, Wq, bq, Wk, bk, Wv, Wo)
    res = run_bass_kernel_spmd(nc, in_maps, core_ids=list(range(8)), trace=False)

    _LAST["mode"] = mode
    _LAST["in_maps"] = in_maps

    extra = (bo.astype(np.float64) + Wo.astype(np.float64) @ bv.astype(np.float64)).astype(np.float32)
    att = np.empty((B, H, S, S), dtype=np.float32)
    out = np.empty((B, S, D), dtype=np.float32)
    for b in range(B):
        r0, r1 = res.results[2 * b], res.results[2 * b + 1]
        att[b, :NHEAD] = r0["ATT"]
        att[b, NHEAD:] = r1["ATT"]
        out[b] = r0["OUTP"] + r1["OUTP"] + extra[None, :]
    return att, out


# revision 4
# speedup vs baseline: 5.6422x; 5.6422x over previous
import contextlib

import numpy as np

import concourse.bacc as bacc
import concourse.mybir as mybir
import concourse.tile as tile
from concourse.bass_utils import run_bass_kernel_spmd

F32 = mybir.dt.float32
F32R = mybir.dt.float32r
AF = mybir.ActivationFunctionType

B, S, D, H, DK = 4, 1024, 1024, 16, 64
NHEAD = 8          # heads per core
GJ = 512           # head-dim columns per core's head group
NEG = -8.0e4       # pre-scale mask addend; *0.125 -> -1e4 -> exp underflows to 0.0


def _build(mode: str, repeat: int = 1):
    nc = bacc.Bacc("TRN2", target_bir_lowering=False)

    XQ = nc.dram_tensor("XQ", [D, S], F32, kind="ExternalInput")
    XK = nc.dram_tensor("XK", [D, S], F32, kind="ExternalInput")
    XV = nc.dram_tensor("XV", [D, S], F32, kind="ExternalInput")
    WQ = nc.dram_tensor("WQ", [D, GJ], F32, kind="ExternalInput")
    WK = nc.dram_tensor("WK", [D, GJ], F32, kind="ExternalInput")
    WV = nc.dram_tensor("WV", [D, GJ], F32, kind="ExternalInput")
    WOT = nc.dram_tensor("WOT", [GJ, D], F32, kind="ExternalInput")
    BQ = nc.dram_tensor("BQ", [1, GJ], F32, kind="ExternalInput")
    BK = nc.dram_tensor("BK", [1, GJ], F32, kind="ExternalInput")
    ONESR = nc.dram_tensor("ONESR", [1, GJ], F32, kind="ExternalInput")
    IDENT = nc.dram_tensor("IDENT", [128, 128], F32, kind="ExternalInput")
    ZEROS = nc.dram_tensor("ZEROS", [128, 128], F32, kind="ExternalInput")
    MASKP = MASKF = None
    if mode == "causal":
        MASKP = nc.dram_tensor("MASKP", [128, 4 * 512], F32, kind="ExternalInput")
    elif mode == "generic":
        MASKF = nc.dram_tensor("MASKF", [S, S], F32, kind="ExternalInput")

    ATT = nc.dram_tensor("ATT", [NHEAD, S, S], F32, kind="ExternalOutput")
    OUTP = nc.dram_tensor("OUTP", [S, D], F32, kind="ExternalOutput")

    causal = mode == "causal"

    with tile.TileContext(nc) as tc:
        with (
            tc.tile_pool(name="pers", bufs=1) as pp,
            tc.tile_pool(name="psum", bufs=1, space="PSUM") as ps,
        ):
            qt = [pp.tile([128, S], F32R, name=f"qt{t}") for t in range(4)]
            kt = [pp.tile([128, S], F32R, name=f"kt{t}") for t in range(4)]
            v1 = [pp.tile([128, GJ], F32R, name=f"v1{j}") for j in range(8)]
            ct = [pp.tile([128, S], F32R, name=f"ct{t}") for t in range(4)]
            wot = [pp.tile([128, S], F32R, name=f"wot{e}") for e in range(4)]
            idr = pp.tile([128, 128], F32R, name="idr")
            idf = pp.tile([128, 128], F32, name="idf")
            zrf = pp.tile([128, 128], F32, name="zrf")
            onesr = pp.tile([1, GJ], F32R, name="onesr")
            bq_t = pp.tile([1, GJ], F32R, name="bq_t")
            bk_t = pp.tile([1, GJ], F32R, name="bk_t")
            mkp = None
            mkf = None
            if causal:
                mkp = pp.tile([128, 4 * 512], F32R, name="mkp")
            elif mode == "generic":
                mkf = [pp.tile([128, S], F32R, name=f"mkf{it}") for it in range(8)]

            for e in range(4):
                nc.sync.dma_start(wot[e], WOT[e * 128:(e + 1) * 128, :].bitcast(F32R))
            nc.sync.dma_start(idr, IDENT[:, :].bitcast(F32R))
            nc.sync.dma_start(idf, IDENT[:, :])
            nc.sync.dma_start(zrf, ZEROS[:, :])
            nc.sync.dma_start(onesr, ONESR[:, :].bitcast(F32R))
            nc.sync.dma_start(bq_t, BQ[:, :].bitcast(F32R))
            nc.sync.dma_start(bk_t, BK[:, :].bitcast(F32R))
            if causal:
                nc.sync.dma_start(mkp, MASKP[:, :].bitcast(F32R))
            elif mode == "generic":
                for it in range(8):
                    nc.sync.dma_start(mkf[it], MASKF[it * 128:(it + 1) * 128, :].bitcast(F32R))

            rep_ctx = (tc.For_i(0, repeat, 1, name="rep") if repeat > 1
                       else contextlib.nullcontext())
            with rep_ctx:
                R = "r_"
                # ---------------- projections ----------------
                with tc.tile_pool(name=f"{R}proj", bufs=1) as sx:
                    for (xd, wd, btile, kind) in (
                        (XQ, WQ, bq_t, "q"), (XK, WK, bk_t, "k"), (XV, WV, None, "v")
                    ):
                        xc, wc = [], []
                        for kc in range(8):
                            xt = sx.tile([128, S], F32R, tag=f"x{kc}", name=f"{R}x_{kind}{kc}")
                            nc.sync.dma_start(xt, xd[kc * 128:(kc + 1) * 128, :].bitcast(F32R))
                            xc.append(xt)
                            wt = sx.tile([128, GJ], F32R, tag=f"w{kc}", name=f"{R}w_{kind}{kc}")
                            nc.sync.dma_start(wt, wd[kc * 128:(kc + 1) * 128, :].bitcast(F32R))
                            wc.append(wt)
                        if kind in ("q", "k"):
                            dst = qt if kind == "q" else kt
                            for jt in range(4):
                                for ih in range(2):
                                    p = ps.tile([128, 512], F32, tag="mm", bufs=3,
                                                name=f"{R}p{kind}{jt}{ih}")
                                    for kc in range(8):
                                        nc.tensor.matmul(
                                            p, wc[kc][:, jt * 128:(jt + 1) * 128],
                                            xc[kc][:, ih * 512:(ih + 1) * 512],
                                            start=(kc == 0), stop=False)
                                    nc.tensor.matmul(
                                        p, btile[0:1, jt * 128:(jt + 1) * 128], onesr,
                                        start=False, stop=True)
                                    nc.scalar.copy(dst[jt][:, ih * 512:(ih + 1) * 512], p)
                        else:
                            for st in range(8):
                                p = ps.tile([128, 512], F32, tag="mm", bufs=3, name=f"{R}pv{st}")
                                for kc in range(8):
                                    nc.tensor.matmul(
                                        p, xc[kc][:, st * 128:(st + 1) * 128], wc[kc],
                                        start=(kc == 0), stop=(kc == 7))
                                nc.scalar.copy(v1[st], p)

                # ---------------- attention ----------------
                with tc.tile_pool(name=f"{R}work", bufs=1) as sw:
                    for h in range(NHEAD):
                        th, off = h // 2, 64 * (h % 2)
                        attb = {}
                        for ih in range(2):
                            for it in range(4 * ih, 4 * ih + 4):
                                jhs = [0, 1] if (it >= 4 or not causal) else [0]
                                accs, exps = [], {}
                                for jh in jhs:
                                    sp = ps.tile([128, 512], F32, tag="mm", bufs=3,
                                                 name=f"{R}s{h}_{it}_{jh}")
                                    qv = it - 4 * jh
                                    mask_add = (mode == "generic") or (causal and 0 <= qv <= 3)
                                    nc.tensor.matmul(
                                        sp,
                                        qt[th][off:off + 64, it * 128:(it + 1) * 128],
                                        kt[th][off:off + 64, jh * 512:(jh + 1) * 512],
                                        start=True, stop=not mask_add)
                                    if mask_add:
                                        rhs = (mkp[:, qv * 512:(qv + 1) * 512] if causal
                                               else mkf[it][:, jh * 512:(jh + 1) * 512])
                                        nc.tensor.matmul(sp, idr, rhs, start=False, stop=True)
                                    ex = sw.tile([128, 512], F32R, tag=f"ex{jh}", bufs=2,
                                                 name=f"{R}ex{h}_{it}_{jh}")
                                    ac = sw.tile([128, 1], F32, tag=f"ac{jh}", bufs=2,
                                                 name=f"{R}ac{h}_{it}_{jh}")
                                    nc.scalar.activation(ex, sp, AF.Exp, bias=0.0, scale=0.125,
                                                         accum_out=ac[:, 0:1])
                                    exps[jh] = ex
                                    accs.append(ac)
                                rec = sw.tile([128, 1], F32, tag="rec", bufs=4, name=f"{R}rc{h}_{it}")
                                if len(accs) == 2:
                                    sm = sw.tile([128, 1], F32, tag="sm", bufs=2, name=f"{R}sm{h}_{it}")
                                    nc.vector.tensor_add(sm, accs[0], accs[1])
                                    nc.vector.reciprocal(rec, sm)
                                else:
                                    nc.vector.reciprocal(rec, accs[0])
                                for jh in jhs:
                                    at = sw.tile([128, 512], F32, tag=f"at{it}_{jh}",
                                                 name=f"{R}at{h}_{it}_{jh}")
                                    nc.scalar.activation(at, exps[jh], AF.Identity, bias=0.0,
                                                         scale=rec[:, 0:1])
                                    attb[(it, jh)] = at
                                    qv = it - 4 * jh
                                    W = (qv + 1) * 128 if (causal and 0 <= qv <= 3) else 512
                                    nc.sync.dma_start(
                                        ATT[h:h + 1, it * 128:(it + 1) * 128, jh * 512:jh * 512 + W],
                                        at[:, 0:W])
                            # ---- AV for this query half ----
                            jts = list(range(4 + 4 * ih)) if causal else list(range(8))
                            up = ps.tile([64, 512], F32, tag="u", bufs=2, name=f"{R}u{h}_{ih}")
                            for jn, jt in enumerate(jts):
                                jh, js = jt // 4, jt % 4
                                tp = ps.tile([128, 512], F32, tag="tr", bufs=2,
                                             name=f"{R}tp{h}_{ih}_{jt}")
                                for idx, it in enumerate(range(4 * ih, 4 * ih + 4)):
                                    if (not causal) or it >= jt:
                                        src = attb[(it, jh)][:, js * 128:(js + 1) * 128]
                                    else:
                                        src = zrf[:, :]
                                    nc.tensor.transpose(tp[:, idx * 128:(idx + 1) * 128], src, idf)
                                aT = sw.tile([128, 512], F32R, tag="aT", bufs=2,
                                             name=f"{R}aT{h}_{ih}_{jt}")
                                nc.vector.tensor_copy(aT, tp)
                                nc.tensor.matmul(up, v1[jt][:, h * 64:(h + 1) * 64], aT,
                                                 start=(jn == 0), stop=(jn == len(jts) - 1))
                            nc.scalar.copy(ct[th][off:off + 64, ih * 512:(ih + 1) * 512], up)

                    # ---------------- output projection ----------------
                    for ic in range(8):
                        for dh in range(2):
                            op = ps.tile([128, 512], F32, tag="mm", bufs=3, name=f"{R}o{ic}_{dh}")
                            for ec in range(4):
                                nc.tensor.matmul(
                                    op, ct[ec][:, ic * 128:(ic + 1) * 128],
                                    wot[ec][:, dh * 512:(dh + 1) * 512],
                                    start=(ec == 0), stop=(ec == 3))
                            ob = sw.tile([128, 512], F32, tag="ob", bufs=2, name=f"{R}ob{ic}_{dh}")
                            nc.scalar.copy(ob, op)
                            nc.sync.dma_start(
                                OUTP[ic * 128:(ic + 1) * 128, dh * 512:(dh + 1) * 512], ob)
    nc.finalize()
    return nc


def _causal_mask_patterns() -> np.ndarray:
    ir = np.arange(128)[:, None]
    jr = np.arange(512)[None, :]
    pats = [np.where(jr > 128 * q + ir, NEG, 0.0).astype(np.float32) for q in range(4)]
    return np.ascontiguousarray(np.concatenate(pats, axis=1))


def _detect_mode(m2: np.ndarray) -> str:
    if not m2.any():
        return "dense"
    if np.array_equal(m2, np.triu(np.ones((S, S), dtype=m2.dtype), 1)):
        return "causal"
    return "generic"


_NC_CACHE: dict = {}
_LAST = {}


def _make_in_maps(mode, query, key, value, m2, Wq, bq, Wk, bk, Wv, Wo):
    ones = np.ones((1, GJ), dtype=np.float32)
    eye = np.eye(128, dtype=np.float32)
    zeros = np.zeros((128, 128), dtype=np.float32)
    maskp = _causal_mask_patterns() if mode == "causal" else None
    maskf = np.ascontiguousarray((m2 * np.float32(NEG)).astype(np.float32)) if mode == "generic" else None
    in_maps = []
    for c in range(8):
        b, g = divmod(c, 2)
        gs = slice(GJ * g, GJ * (g + 1))
        im = {
            "XQ": np.ascontiguousarray(query[b].T.astype(np.float32, copy=False)),
            "XK": np.ascontiguousarray(key[b].T.astype(np.float32, copy=False)),
            "XV": np.ascontiguousarray(value[b].T.astype(np.float32, copy=False)),
            "WQ": np.ascontiguousarray(Wq[gs, :].T.astype(np.float32, copy=False)),
            "WK": np.ascontiguousarray(Wk[gs, :].T.astype(np.float32, copy=False)),
            "WV": np.ascontiguousarray(Wv[gs, :].T.astype(np.float32, copy=False)),
            "WOT": np.ascontiguousarray(Wo[:, gs].T.astype(np.float32, copy=False)),
            "BQ": np.ascontiguousarray(bq[gs].reshape(1, GJ).astype(np.float32, copy=False)),
            "BK": np.ascontiguousarray(bk[gs].reshape(1, GJ).astype(np.float32, copy=False)),
            "ONESR": ones,
            "IDENT": eye,
            "ZEROS": zeros,
        }
        if mode == "causal":
            im["MASKP"] = maskp
        elif mode == "generic":
            im["MASKF"] = maskf
        in_maps.append(im)
    return in_maps


def kernel(query, key, value, mask, Wq, bq, Wk, bk, Wv, bv, Wo, bo):
    query = np.asarray(query)
    key = np.asarray(key)
    value = np.asarray(value)
    mask = np.asarray(mask)
    Wq, bq = np.asarray(Wq), np.asarray(bq)
    Wk, bk = np.asarray(Wk), np.asarray(bk)
    Wv, bv = np.asarray(Wv), np.asarray(bv)
    Wo, bo = np.asarray(Wo), np.asarray(bo)

    m2 = mask.reshape(S, S).astype(np.float32, copy=False)
    mode = _detect_mode(m2)
    if mode not in _NC_CACHE:
        _NC_CACHE[mode] = _build(mode)
    nc = _NC_CACHE[mode]

    in_maps = _make_in_maps(mode, query, key, value, m2, Wq, bq, Wk, bk, Wv, Wo)
    res = run_bass_kernel_spmd(nc, in_maps, core_ids=list(range(8)), trace=False)

    _LAST["mode"] = mode
    _LAST["in_maps"] = in_maps

    extra = (bo.astype(np.float64) + Wo.astype(np.float64) @ bv.astype(np.float64)).astype(np.float32)
    att = np.empty((B, H, S, S), dtype=np.float32)
    out = np.empty((B, S, D), dtype=np.float32)
    for b in range(B):
        r0, r1 = res.results[2 * b], res.results[2 * b + 1]
        att[b, :NHEAD] = r0["ATT"]
        att[b, NHEAD:] = r1["ATT"]
        out[b] = r0["OUTP"] + r1["OUTP"] + extra[None, :]
    return att, out
